# revision 26
# baseline (speedup 1.0000x reference)
"""Trainium2 Bass kernel for BertTempRel-style span-mean + MLP + softmax.

Reference computation (per batch row b of x[B, S, D]):
    e1 = mean(x[b, lo1:hi1, :]),  e2 = mean(x[b, lo2:hi2, :]),  cls = x[b, 0, :]
    (hi = max(hi, lo+1) empty-span guard)
    feat = concat([e1, e2, cls])            # [3D]
    out = softmax(relu(relu(feat@W1+b1)@W2+b2)@W3+b3)

Strategy: pure data-parallel over 8 NeuronCores (128 batch rows each),
with host-side LPT balancing of per-row HBM traffic across cores
(_row_assignment). Per core, only the rows in span1 ∪ span2 ∪ {CLS} are
read from HBM (~55% of x) via per-row dma_gather (CounterMachine SWDGE)
with a compacted index list, spread over 4 SWDGE queues; see the
build_program comment. The span means are computed on the TensorEngine:
for each b, the tiny 0/1 span masks (in compacted coordinates, plus a
one-hot row for CLS) form the *stationary* operand [128s, 3] and the x
tile [128s, D] is the *moving* operand, accumulating [3, D] span sums in
PSUM over the 4 s-chunks (float32r mode: 1 cycle/row at N>=256). PSUM is
evacuated with a fused 1/count scale, transposed back to [d, b] layout via
tiny PE transposes, and the whole 128-row MLP runs as a handful of
matmuls at the end.
"""

import sys

if "/opt/trn_rl_repo" not in sys.path:
    sys.path.insert(0, "/opt/trn_rl_repo")

import numpy as np

from concourse import bacc, bass, mybir, tile
from concourse.bass_utils import run_bass_kernel_spmd
from concourse.masks import make_identity

F32 = mybir.dt.float32
F32R = mybir.dt.float32r
I32 = mybir.dt.int32
I16 = mybir.dt.int16
OP = mybir.AluOpType
AF = mybir.ActivationFunctionType

N_CORES = 8
B_FULL, S, D = 1024, 512, 768
H1, H2, H3 = 256, 64, 4
BPC = B_FULL // N_CORES  # batch rows per core (128)
BPD = 2                  # batch rows loaded per DMA


def build_program(bpc=BPC, s=S, d=D, h1=H1, h2=H2, h3=H3, bpd=BPD, rep=1,
                  xbufs=3, gather=True, nq=4, dbg=False):
    # gather=True replaces the dense x stream with per-row dma_gather
    # (InstDMAGatherAnt, the CounterMachine SWDGE gather): per batch row
    # only the rows of span1 ∪ span2 ∪ {CLS} are read from HBM (~55% of
    # bytes on random spans). The gathered list is COMPACTED:
    #   position 0            -> row 0 (CLS)
    #   positions [1, 1+L1)   -> interval I1 = [loA, loA+L1)
    #   positions [1+L1, n)   -> interval I2 = [loB', loB'+L2)
    #   positions [n, 512)    -> idx -1 (ignored tail, no transfer)
    # where I1/I2 are span1 ∪ span2 merged into 1-2 disjoint intervals.
    # The span masks are expressed in compacted coordinates, so the PE
    # mask-matmul machinery is unchanged (CLS one-hot stays position 0).
    # The first xbufs*bpd rows per core force the identity list (n=512,
    # full stream) so every xb pool buffer holds finite data before any
    # tail is left stale; make_in_maps parks the longest-union rows there.
    #
    # (Tried and rejected on HW: per-row indirect_dma_start = ~208 ns per
    # descriptor on the generic qPoolDynamic SWDGE path -> 24x SLOWER than
    # dense; HWDGE cond=/dynamic-register-offset DMAs die with an opaque
    # INTERNAL error at device execution through this axon deployment --
    # probe.py variant 'dyn' reproduces it with a minimal program.)
    # skip=True predicates each 64-row slab DMA on its span masks being
    # nonzero (~24% fewer bytes on this data; compute unchanged since zero
    # masks nullify stale slabs). The packed-word form below (one value_load
    # per b, per-slab conds via (w >> k) & 1) clears both earlier compile
    # blockers (register exhaustion from hoisted per-slab loads; missing DGE
    # sync info under tile_critical) and passes walrus, but the resulting
    # NEFF fails at device execution through this axon deployment with an
    # opaque INTERNAL error. Left disabled; revisit where NRT diagnostics
    # are available.
    """Emit the per-core Bass/Tile program. All 8 cores run it SPMD."""
    sc = s // 128          # s-chunks
    dh = d // 2            # moving free-dim per span matmul
    nd = d // 128          # d-chunks of 128
    nf = 3 * d // 128      # feature chunks of 128
    nh1 = h1 // 128

    nc = bacc.Bacc("TRN2", target_bir_lowering=False, debug=False,
                   num_devices=N_CORES, num_swdge_queues=nq)

    x_d = nc.dram_tensor("x", [bpc, s, d], F32R, kind="ExternalInput")
    e1_d = nc.dram_tensor("e1", [bpc, 2], I32, kind="ExternalInput")
    e2_d = nc.dram_tensor("e2", [bpc, 2], I32, kind="ExternalInput")
    w1_d = nc.dram_tensor("W1", [nf, 128, h1], F32, kind="ExternalInput")
    b1_d = nc.dram_tensor("b1", [1, h1], F32, kind="ExternalInput")
    w2_d = nc.dram_tensor("W2", [nh1, 128, h2], F32, kind="ExternalInput")
    b2_d = nc.dram_tensor("b2", [1, h2], F32, kind="ExternalInput")
    w3_d = nc.dram_tensor("W3", [h2, h3], F32, kind="ExternalInput")
    b3_d = nc.dram_tensor("b3", [1, h3], F32, kind="ExternalInput")
    out_d = nc.dram_tensor("out", [bpc, h3], F32, kind="ExternalOutput")
    if dbg:
        dbg_wr = nc.dram_tensor("dbg_wr", [128, bpc, 32], I32,
                                kind="ExternalOutput")
        dbg_cnt = nc.dram_tensor("dbg_cnt", [bpc, 1], I32,
                                 kind="ExternalOutput")
        dbg_idx = nc.dram_tensor("dbg_idx", [bpc, s], F32,
                                 kind="ExternalOutput")
        dbg_cb = nc.dram_tensor("dbg_cb", [bpc, 4], F32,
                                kind="ExternalOutput")

    with tile.TileContext(nc) as tc:
        with tc.tile_pool(name="const", bufs=1) as const:
            ident = const.tile([128, 128], F32)
            make_identity(nc, ident[:])

            w1 = const.tile([128, nf, h1], F32)
            nc.sync.dma_start(w1[:], w1_d.ap().rearrange("p k h -> k p h"))
            w2 = const.tile([128, nh1, h2], F32)
            nc.sync.dma_start(w2[:], w2_d.ap().rearrange("p k h -> k p h"))
            w3 = const.tile([h2, h3], F32)
            nc.sync.dma_start(w3[:], w3_d.ap()[:])
            b1r = const.tile([1, h1], F32)
            nc.sync.dma_start(b1r[:], b1_d.ap()[:])
            b2r = const.tile([1, h2], F32)
            nc.sync.dma_start(b2r[:], b2_d.ap()[:])
            b3r = const.tile([1, h3], F32)
            nc.sync.dma_start(b3r[:], b3_d.ap()[:])
            ones = const.tile([1, 128], F32)
            nc.vector.memset(ones[:], 1.0)

            if gather:
                # sel[k, u, P] = 1 iff k == 16u + P%16 -- stationary matrices
                # that turn an s-transposed index column block into the
                # "wrapped in 16 partitions, replicated 8x" layout dma_gather
                # wants for its index tile.
                pm16_i = const.tile([128, 128], I32)
                nc.gpsimd.iota(pm16_i[:], pattern=[[0, 8], [1, 16]], base=0,
                               channel_multiplier=0)
                kk_i = const.tile([128, 128], I32)
                nc.gpsimd.iota(kk_i[:], pattern=[[0, 128]], base=0,
                               channel_multiplier=1)
                pm16_f = const.tile([128, 128], F32)
                nc.vector.tensor_copy(pm16_f[:], pm16_i[:])
                kk_f = const.tile([128, 128], F32)
                nc.vector.tensor_copy(kk_f[:], kk_i[:])
                sel = const.tile([128, 8, 128], F32)
                selt = const.tile([128, 128], F32)
                for u in range(8):
                    nc.vector.tensor_scalar(selt[:], pm16_f[:], float(16 * u),
                                            None, OP.add)
                    nc.vector.tensor_tensor(sel[:, u, :], kk_f[:], selt[:],
                                            OP.is_equal)

            for _rep in range(rep):
                # ---- span bounds, counts, reciprocal counts ([b, *] layout) ----
                sp_i = const.tile([bpc, 4], I32)
                nc.sync.dma_start(sp_i[:, 0:2], e1_d.ap()[:])
                nc.sync.dma_start(sp_i[:, 2:4], e2_d.ap()[:])
                sp_f = const.tile([bpc, 4], F32)
                nc.vector.tensor_copy(sp_f[:], sp_i[:])

                bounds = const.tile([bpc, 4], F32)  # lo1, hi1, lo2, hi2 (guarded)
                rp = const.tile([bpc, 3], F32)      # 1/cnt1, 1/cnt2, 1.0
                cnt = const.tile([bpc, 2], F32)
                for j in range(2):
                    lo = sp_f[:, 2 * j:2 * j + 1]
                    hi_raw = sp_f[:, 2 * j + 1:2 * j + 2]
                    lo_out = bounds[:, 2 * j:2 * j + 1]
                    hi_out = bounds[:, 2 * j + 1:2 * j + 2]
                    nc.vector.tensor_copy(lo_out, lo)
                    # hi = max(hi_raw, lo+1); cnt = hi - lo; rp = 1/cnt
                    nc.vector.tensor_scalar(hi_out, lo, 1.0, None, OP.add)
                    nc.vector.tensor_tensor(hi_out, hi_raw, hi_out, OP.max)
                    nc.vector.tensor_tensor(cnt[:, j:j + 1], hi_out, lo_out,
                                            OP.subtract)
                    nc.vector.reciprocal(rp[:, j:j + 1], cnt[:, j:j + 1])
                nc.vector.memset(rp[:, 2:3], 1.0)

                # ---- masks in [b, s] layout ----
                iota_i = const.tile([bpc, s], I32)
                nc.gpsimd.iota(iota_i[:], pattern=[[1, s]], base=0,
                               channel_multiplier=0)
                iota_f = const.tile([bpc, s], F32)
                nc.vector.tensor_copy(iota_f[:], iota_i[:])

                if gather:
                    # ---- merge spans into 1-2 disjoint intervals ----
                    # A = span with smaller lo, B = other. merged iff
                    # loB <= hiA -> I1 = [loA, max(hiA,hiB)), I2 empty;
                    # else I1 = A, I2 = B. All [bpc, 1] f32 exact ints.
                    lo1c, hi1c = bounds[:, 0:1], bounds[:, 1:2]
                    lo2c, hi2c = bounds[:, 2:3], bounds[:, 3:4]
                    sc16 = const.tile([bpc, 16], F32)
                    c12, loA, hiA, loB = (sc16[:, k:k + 1] for k in range(4))
                    hiB, mrg, mx, L1 = (sc16[:, k:k + 1] for k in range(4, 8))
                    L2, ncol, L1p1, loAm1 = (sc16[:, k:k + 1]
                                             for k in range(8, 12))
                    loBm, tmp, tmp2, c12i = (sc16[:, k:k + 1]
                                             for k in range(12, 16))
                    vec = nc.vector
                    vec.tensor_tensor(c12[:], lo2c, lo1c, OP.is_ge)
                    vec.tensor_tensor(loA[:], lo1c, lo2c, OP.min)
                    vec.tensor_tensor(loB[:], lo1c, lo2c, OP.max)
                    vec.tensor_tensor(tmp[:], hi1c, hi2c, OP.subtract)
                    vec.scalar_tensor_tensor(hiA[:], c12[:], tmp[:], hi2c,
                                             OP.mult, OP.add)
                    vec.tensor_tensor(tmp2[:], hi1c, hi2c, OP.add)
                    vec.tensor_tensor(hiB[:], tmp2[:], hiA[:], OP.subtract)
                    # first xbufs*bpd rows: force identity list (full
                    # stream) so every xb pool buffer starts finite
                    ndr = xbufs * bpd
                    vec.memset(sc16[0:ndr, 1:2], 1.0)        # loA
                    vec.memset(sc16[0:ndr, 2:3], float(s))   # hiA
                    vec.tensor_tensor(mrg[:], hiA[:], loB[:], OP.is_ge)
                    vec.tensor_tensor(mx[:], hiA[:], hiB[:], OP.max)
                    vec.tensor_tensor(tmp[:], mx[:], hiA[:], OP.subtract)
                    vec.scalar_tensor_tensor(tmp2[:], mrg[:], tmp[:], hiA[:],
                                             OP.mult, OP.add)  # end of I1
                    vec.tensor_tensor(L1[:], tmp2[:], loA[:], OP.subtract)
                    vec.tensor_tensor(tmp[:], hiB[:], loB[:], OP.subtract)
                    vec.tensor_tensor(tmp2[:], mrg[:], tmp[:], OP.mult)
                    vec.tensor_tensor(L2[:], tmp[:], tmp2[:], OP.subtract)
                    vec.tensor_tensor(tmp[:], L1[:], L2[:], OP.add)
                    vec.tensor_scalar(ncol[:], tmp[:], 1.0, None, OP.add)
                    vec.tensor_scalar(L1p1[:], L1[:], 1.0, None, OP.add)
                    vec.tensor_scalar(loAm1[:], loA[:], -1.0, None, OP.add)
                    vec.tensor_scalar(tmp[:], loB[:], -1.0, None, OP.add)
                    vec.tensor_tensor(loBm[:], tmp[:], L1[:], OP.subtract)
                    vec.tensor_scalar(c12i[:], c12[:], -1.0, 1.0, OP.mult,
                                      OP.add)
                    # per-span bounds in compacted coordinates:
                    # base_j = inI1 ? 1 + lo_j - loA : 1 + L1  (lo_j == loB)
                    cb = const.tile([bpc, 4], F32)
                    for j, isA in ((0, c12), (1, c12i)):
                        loj = bounds[:, 2 * j:2 * j + 1]
                        vec.tensor_tensor(tmp[:], mrg[:], isA[:], OP.max)
                        vec.tensor_tensor(tmp2[:], loj, loA[:], OP.subtract)
                        vec.tensor_tensor(tmp2[:], tmp2[:], L1[:], OP.subtract)
                        vec.scalar_tensor_tensor(cb[:, 2 * j:2 * j + 1],
                                                 tmp[:], tmp2[:], L1p1[:],
                                                 OP.mult, OP.add)
                        vec.tensor_tensor(cb[:, 2 * j + 1:2 * j + 2],
                                          cb[:, 2 * j:2 * j + 1],
                                          cnt[:, j:j + 1], OP.add)
                    cnt_i = const.tile([bpc, 1], I32)
                    nc.vector.tensor_copy(cnt_i[:], ncol[:])

                masks = const.tile([bpc, 3, s], F32)
                ge = const.tile([bpc, s], F32)
                mb = cb if gather else bounds
                for j in range(2):
                    lo = mb[:, 2 * j:2 * j + 1]
                    hi = mb[:, 2 * j + 1:2 * j + 2]
                    nc.vector.tensor_scalar(ge[:], iota_f[:], lo, None, OP.is_ge)
                    nc.vector.scalar_tensor_tensor(masks[:, j, :], iota_f[:], hi,
                                                   ge[:], OP.is_lt, OP.mult)
                # CLS one-hot "mask": 1.0 at position 0 (compacted position 0
                # always holds row 0)
                nc.vector.tensor_scalar(masks[:, 2, :], iota_f[:], 0.0, None,
                                        OP.is_equal)

                if gather:
                    # ---- compacted gather list, [b, position] layout ----
                    # idx(i) = 0 at i=0; loA+i-1 on [1,1+L1);
                    # loB+i-1-L1 on [1+L1, n); -1 tail.
                    idxbs = const.tile([bpc, s], F32)
                    ge1 = const.tile([bpc, s], F32)
                    lt1 = const.tile([bpc, s], F32)
                    lt2 = const.tile([bpc, s], F32)
                    sg1 = const.tile([bpc, s], F32)
                    sg2 = const.tile([bpc, s], F32)
                    a12 = const.tile([bpc, s], F32)
                    vec.tensor_scalar(ge1[:], iota_f[:], 1.0, None, OP.is_ge)
                    vec.tensor_scalar(lt1[:], iota_f[:], L1p1[:], None,
                                      OP.is_lt)
                    vec.tensor_tensor(sg1[:], lt1[:], ge1[:], OP.mult)
                    vec.tensor_scalar(lt2[:], iota_f[:], ncol[:], None,
                                      OP.is_lt)
                    vec.tensor_tensor(sg2[:], lt1[:], lt2[:], OP.mult)
                    vec.tensor_tensor(sg2[:], lt2[:], sg2[:], OP.subtract)
                    vec.tensor_scalar(a12[:], iota_f[:], loAm1[:], None, OP.add)
                    vec.tensor_tensor(a12[:], sg1[:], a12[:], OP.mult)
                    vec.tensor_scalar(idxbs[:], iota_f[:], loBm[:], None,
                                      OP.add)
                    vec.tensor_tensor(idxbs[:], sg2[:], idxbs[:], OP.mult)
                    vec.tensor_tensor(idxbs[:], idxbs[:], a12[:], OP.add)
                    vec.tensor_tensor(sg1[:], sg1[:], sg2[:], OP.add)
                    vec.tensor_tensor(idxbs[:], idxbs[:], sg1[:], OP.add)
                    vec.tensor_tensor(idxbs[:], idxbs[:], masks[:, 2, :],
                                      OP.add)
                    vec.tensor_scalar(idxbs[:], idxbs[:], -1.0, None, OP.add)

                # ---- transpose masks/scales to [s, b] / [3, b] layouts ----
                # mt[s_p, c, b, m]: stationary operand source; m: e1, e2, cls.
                mt = const.tile([128, sc, bpc, 3], F32R)
                scl = const.tile([3, bpc], F32)
                if gather:
                    # wrapped[P, b, t] = idx_b(16t + P%16), int16 -- the
                    # dma_gather index layout (16-wrapped, replicated 8x
                    # down the partitions for the 8 Q7 cores)
                    wrapped = const.tile([128, bpc, 32], I16)
                with tc.tile_pool(name="p0psum", bufs=2, space="PSUM") as p0p, \
                     tc.tile_pool(name="itp", bufs=2) as itp:
                    for c in range(sc):
                        for j in range(3):
                            tp = p0p.tile([128, bpc], F32, tag="tp")
                            nc.tensor.transpose(tp[:], masks[:, j, bass.ts(c, 128)],
                                                ident[0:bpc, 0:bpc])
                            nc.vector.tensor_copy(mt[:, c, :, j], tp[:])
                        if gather:
                            tpx = p0p.tile([128, bpc], F32, tag="tp")
                            nc.tensor.transpose(tpx[:], idxbs[:, bass.ts(c, 128)],
                                                ident[0:bpc, 0:bpc])
                            tsb = itp.tile([128, bpc], F32, tag="tsb")
                            nc.vector.tensor_copy(tsb[:], tpx[:])
                            for u in range(8):
                                pw = p0p.tile([128, bpc], F32, tag="tp")
                                nc.tensor.matmul(pw[:], sel[:, u, :], tsb[:],
                                                 start=True, stop=True)
                                if u % 2 == 0:
                                    nc.vector.tensor_copy(
                                        wrapped[:, :, 8 * c + u], pw[:])
                                else:
                                    nc.scalar.copy(
                                        wrapped[:, :, 8 * c + u], pw[:])
                    tps = p0p.tile([3, bpc], F32, tag="tps")
                    nc.tensor.transpose(tps[:], rp[:], ident[0:bpc, 0:bpc])
                    nc.vector.tensor_copy(scl[:], tps[:])

                if gather and dbg:
                    wr32 = const.tile([128, bpc, 32], I32)
                    nc.vector.tensor_copy(wr32[:], wrapped[:])
                    nc.sync.dma_start(dbg_wr.ap()[:], wr32[:])
                    nc.sync.dma_start(dbg_cnt.ap()[:], cnt_i[:])
                    nc.sync.dma_start(dbg_idx.ap()[:], idxbs[:])
                    nc.sync.dma_start(dbg_cb.ap()[:], cb[:])

                # packT[d_p, dc, b, m]: transposed scaled span sums / cls.
                packT = const.tile([128, nd, bpc, 3], F32)

                # ---- main loop: stream x, accumulate span sums on PE ----
                with tc.tile_pool(name="xp", bufs=xbufs) as xp, \
                     tc.tile_pool(name="stg", bufs=4) as stg, \
                     tc.tile_pool(name="sps0", bufs=2, space="PSUM") as sps0, \
                     tc.tile_pool(name="sps1", bufs=2, space="PSUM") as sps1, \
                     tc.tile_pool(name="ptp", bufs=2, space="PSUM") as ptp:
                    for i in range(bpc // bpd):
                        xb = xp.tile([128, bpd, sc, d], F32R, tag="xb")
                        if gather:
                            for j in range(bpd):
                                b = bpd * i + j
                                cr = nc.gpsimd.value_load(cnt_i[b:b + 1, 0:1])
                                nc.gpsimd.dma_gather(
                                    out_ap=xb[:, j, :, :],
                                    in_ap=x_d.ap()[b],
                                    idxs_ap=wrapped[:, b, :],
                                    num_idxs=s,
                                    num_idxs_reg=cr,
                                    elem_size=d,
                                    queue_num=b % nq)
                        else:
                            dma_eng = nc.sync if i % 2 == 0 else nc.scalar
                            dma_eng.dma_start(
                                xb[:],
                                x_d.ap()[bpd * i:bpd * (i + 1)].rearrange(
                                    "b (c p) d -> p b c d", p=128))
                        for j in range(bpd):
                            b = bpd * i + j
                            ps0 = sps0.tile([3, dh], F32, tag="ps0")
                            ps1 = sps1.tile([3, dh], F32, tag="ps1")
                            for c in range(sc):
                                lhsT = mt[:, c, b, :]
                                nc.tensor.matmul(ps0[:], lhsT,
                                                 xb[:, j, c, 0:dh],
                                                 start=(c == 0), stop=(c == sc - 1))
                                nc.tensor.matmul(ps1[:], lhsT,
                                                 xb[:, j, c, dh:d],
                                                 start=(c == 0), stop=(c == sc - 1))
                            # evacuate + scale by 1/cnt (split across DVE/ACT)
                            sg = stg.tile([3, d], F32, tag="sg")
                            nc.vector.tensor_scalar(sg[:, 0:dh], ps0[:],
                                                    scl[:, b:b + 1], None, OP.mult)
                            nc.scalar.mul(sg[:, dh:d], ps1[:], scl[:, b:b + 1])
                            # transpose [3, d] -> nd x [128, 3] columns of packT
                            for dc in range(nd):
                                pt = ptp.tile([128, 3], F32, tag="pt")
                                nc.tensor.transpose(pt[:], sg[:, bass.ts(dc, 128)],
                                                    ident[0:3, 0:3])
                                if dc % 2 == 0:
                                    nc.vector.tensor_copy(packT[:, dc, b, :], pt[:])
                                else:
                                    nc.scalar.copy(packT[:, dc, b, :], pt[:])

                # ---- de-interleave features: featT[f_p, p, b] ----
                featT = const.tile([128, nf, bpc], F32)
                for m in range(3):
                    for dc in range(nd):
                        nc.vector.tensor_copy(featT[:, m * nd + dc, :],
                                              packT[:, dc, :, m])

                # ---- MLP + softmax over all bpc rows at once ----
                h1s = const.tile([bpc, h1], F32)
                h1T = const.tile([128, nh1, bpc], F32)
                h2s = const.tile([bpc, h2], F32)
                h2T = const.tile([h2, bpc], F32)
                probs = const.tile([bpc, h3], F32)
                mx = const.tile([bpc, 1], F32)
                ex = const.tile([bpc, h3], F32)
                sm = const.tile([bpc, 1], F32)
                rc = const.tile([bpc, 1], F32)

                with tc.tile_pool(name="mlpp", bufs=1, space="PSUM") as mp:
                    h1p = mp.tile([bpc, h1], F32, tag="h1p")
                    for p in range(nf):
                        nc.tensor.matmul(h1p[:], featT[:, p, :], w1[:, p, :],
                                         start=(p == 0), stop=False)
                    nc.tensor.matmul(h1p[:], ones[0:1, 0:bpc], b1r[:],
                                     start=False, stop=True)
                    nc.scalar.activation(h1s[:], h1p[:], AF.Relu)

                    for k in range(nh1):
                        tp1 = mp.tile([128, bpc], F32, tag="tp1")
                        nc.tensor.transpose(tp1[:], h1s[:, bass.ts(k, 128)],
                                            ident[0:bpc, 0:bpc])
                        nc.vector.tensor_copy(h1T[:, k, :], tp1[:])

                    h2p = mp.tile([bpc, h2], F32, tag="h2p")
                    for k in range(nh1):
                        nc.tensor.matmul(h2p[:], h1T[:, k, :], w2[:, k, :],
                                         start=(k == 0), stop=False)
                    nc.tensor.matmul(h2p[:], ones[0:1, 0:bpc], b2r[:],
                                     start=False, stop=True)
                    nc.scalar.activation(h2s[:], h2p[:], AF.Relu)

                    tp2 = mp.tile([h2, bpc], F32, tag="tp2")
                    nc.tensor.transpose(tp2[:], h2s[:], ident[0:bpc, 0:bpc])
                    nc.vector.tensor_copy(h2T[:], tp2[:])

                    h3p = mp.tile([bpc, h3], F32, tag="h3p")
                    nc.tensor.matmul(h3p[:], h2T[:], w3[:], start=True, stop=False)
                    nc.tensor.matmul(h3p[:], ones[0:1, 0:bpc], b3r[:],
                                     start=False, stop=True)

                    # softmax along the 4 logits
                    nc.vector.tensor_reduce(mx[:], h3p[:], mybir.AxisListType.X,
                                            OP.max, negate=True)
                    nc.scalar.activation(ex[:], h3p[:], AF.Exp, bias=mx[:],
                                         scale=1.0)
                    nc.vector.tensor_reduce(sm[:], ex[:], mybir.AxisListType.X,
                                            OP.add)
                    nc.vector.reciprocal(rc[:], sm[:])
                    nc.vector.tensor_scalar(probs[:], ex[:], rc[:], None, OP.mult)

                nc.sync.dma_start(out_d.ap()[:], probs[:])

    nc.compile()
    return nc


_NC_CACHE = {}


def _get_program():
    if "nc" not in _NC_CACHE:
        _NC_CACHE["nc"] = build_program()
    return _NC_CACHE["nc"]


def _row_assignment(e1, e2, xbufs=3, bpd=BPD):
    """Balanced data-parallel row assignment (host-side sharding policy).

    The kernel reads only span1 ∪ span2 ∪ {CLS} rows per batch element,
    except the first xbufs*bpd rows per core which stream densely (S rows).
    Assign the globally longest unions to those dense slots (their dense
    cost is the same 512 rows regardless of content, so parking long
    unions there wastes the least), then LPT-balance the rest so every
    core moves nearly equal HBM bytes. Returns perm[core, row_in_core].
    """
    e1 = np.asarray(e1, dtype=np.int64)
    e2 = np.asarray(e2, dtype=np.int64)
    b = e1.shape[0]
    lo1, hi1 = e1[:, 0], np.maximum(e1[:, 1], e1[:, 0] + 1)
    lo2, hi2 = e2[:, 0], np.maximum(e2[:, 1], e2[:, 0] + 1)
    # |union of two [lo,hi) intervals| (+1 CLS row when not covered)
    inter = np.maximum(
        0, np.minimum(hi1, hi2) - np.maximum(lo1, lo2))
    union = (hi1 - lo1) + (hi2 - lo2) - inter
    union = union + (np.minimum(lo1, lo2) > 0)
    order = np.argsort(-union, kind="stable")

    n_dense = xbufs * bpd
    perm = np.empty((N_CORES, b // N_CORES), dtype=np.int64)
    # longest unions -> dense slots, dealt round-robin
    head = order[: n_dense * N_CORES]
    for c in range(N_CORES):
        perm[c, :n_dense] = head[c::N_CORES]
    # LPT greedy for the rest
    rest = order[n_dense * N_CORES:]
    loads = np.zeros(N_CORES, dtype=np.int64)
    counts = np.full(N_CORES, n_dense, dtype=np.int64)
    cap = b // N_CORES
    for r in rest:
        open_cores = np.flatnonzero(counts < cap)
        c = open_cores[np.argmin(loads[open_cores])]
        perm[c, counts[c]] = r
        counts[c] += 1
        loads[c] += union[r]
    return perm


def make_in_maps(inputs):
    x = np.ascontiguousarray(np.asarray(inputs["x"], dtype=np.float32))
    e1 = np.ascontiguousarray(np.asarray(inputs["e1_span"], dtype=np.int32))
    e2 = np.ascontiguousarray(np.asarray(inputs["e2_span"], dtype=np.int32))
    w1 = np.ascontiguousarray(
        np.asarray(inputs["W1"], dtype=np.float32).reshape(3 * D // 128, 128, H1))
    b1 = np.asarray(inputs["b1"], dtype=np.float32).reshape(1, H1)
    w2 = np.ascontiguousarray(
        np.asarray(inputs["W2"], dtype=np.float32).reshape(H1 // 128, 128, H2))
    b2 = np.asarray(inputs["b2"], dtype=np.float32).reshape(1, H2)
    w3 = np.ascontiguousarray(np.asarray(inputs["W3"], dtype=np.float32))
    b3 = np.asarray(inputs["b3"], dtype=np.float32).reshape(1, H3)

    perm = _row_assignment(e1, e2)
    in_maps = []
    for c in range(N_CORES):
        sl = perm[c]
        in_maps.append({
            "x": np.ascontiguousarray(x[sl]),
            "e1": np.ascontiguousarray(e1[sl]),
            "e2": np.ascontiguousarray(e2[sl]),
            "W1": w1, "b1": b1, "W2": w2, "b2": b2, "W3": w3, "b3": b3,
        })
    return in_maps


def kernel(**inputs) -> np.ndarray:
    nc = _get_program()
    res = run_bass_kernel_spmd(nc, make_in_maps(inputs),
                               core_ids=list(range(N_CORES)))
    perm = _row_assignment(inputs["e1_span"], inputs["e2_span"])
    out = np.empty((B_FULL, H3), dtype=np.float32)
    for c in range(N_CORES):
        out[perm[c]] = res.results[c]["out"]
    return out

